# revision 16
# baseline (speedup 1.0000x reference)
"""KNN topological BCE loss (N=8192, D=128, k=8) on 8 Trainium2 NeuronCores.

Loss decomposition (validated to ~2e-6 rel against the torch/jax reference):
  loss_ij = 100*(t_ij + A_ij*(1-2 t_ij))
  mean loss = 100*(S_t + S_Au)/N^2,  S_t = sum(t),  S_Au = sum_{A_ij=1} (1-2 t_ij)
where A is the symmetrized k=8 NN adjacency: A = D ∪ D^T for the directed
edge set D = {(i, j) : j in knn_8(i)}.

A depends only on Z; t enters only through S_t (a full sum) and ~131k
gathered entries on A's support.  So the device never sees target_adj
(256MB): each core uploads its 1024x128 fp8-e4m3 shard of Z (1MB total
H2D), transposes it (fp8 matmul vs identity), AllGathers Z^T on-device
as bf16, computes its 1024x8192 block of v_ij = 2 z_i.z_j - |z_j|^2
(order-reversed squared distance) in bf16, masks the diagonal, and
extracts the top-8 values+indices per row with the DVE max8/max_index
instructions.  Only the [1024, 8x8] uint16 index block (16KB/core)
returns to the host.  The host computes S_t in a background thread
(overlapped with the device round-trip) and the sparse symmetrized
gather-sum with numpy.  fp8 quantization of Z perturbs the loss by
~2e-6 rel (tolerance 2e-2): only near-tied neighbor picks can differ,
and each edge is worth 1.5e-6 of loss.
"""
import sys
import threading

sys.path.insert(0, "/opt/trn_rl_repo")

import numpy as np
import ml_dtypes

import concourse.bass as bass
import concourse.mybir as mybir
import concourse.tile as tile
from concourse import bacc
from concourse.bass import ds, ts
from concourse.masks import make_identity

F32 = mybir.dt.float32
BF16 = mybir.dt.bfloat16
F8 = mybir.dt.float8e4
U16 = mybir.dt.uint16
AF = mybir.ActivationFunctionType
OP = mybir.AluOpType

N = 8192
D = 128
K = 8
NCORES = 8
R = N // NCORES          # 1024 rows per core
NSTRIP = R // 128        # 8 strips of 128 rows per core
CT = 512                 # psum col tile
NCT = N // CT            # 16
BIG = 65536.0

_CACHE = {}


def build():
    nc = bacc.Bacc("TRN2", target_bir_lowering=False, debug=False,
                   num_devices=NCORES)

    zs = nc.declare_dram_parameter("zs", [R, D], F8, isOutput=False)
    oidx = nc.declare_dram_parameter("oidx", [128, NSTRIP * K], U16,
                                     isOutput=True)

    cc_in = nc.dram_tensor("cc_in", [D, R], BF16)
    cc_out = nc.dram_tensor("cc_out", [NCORES * D, R], BF16,
                            addr_space="Shared")

    with tile.TileContext(nc) as tc:
        with tc.tile_pool(name="const", bufs=1) as const, \
             tc.tile_pool(name="stream", bufs=2) as stream, \
             tc.tile_pool(name="vpool", bufs=2) as vpool, \
             tc.tile_pool(name="work", bufs=2) as work, \
             tc.tile_pool(name="psum", bufs=4, space="PSUM") as psum, \
             tc.tile_pool(name="psmall", bufs=2, space="PSUM") as psmall:

            # ---------- constants ----------
            ones1 = const.tile([1, 128], BF16)
            nc.gpsimd.memset(ones1[:, :], 1.0)
            ones_col = const.tile([128, 1], BF16)
            nc.gpsimd.memset(ones_col[:, :], 1.0)
            ident = const.tile([128, 128], BF16)
            make_identity(nc, ident[:, :])
            mbig = const.tile([128, 128], BF16)
            nc.vector.tensor_scalar_mul(mbig[:, :], ident[:, :], -BIG)
            ident8 = const.tile([128, 128], F8)
            nc.vector.tensor_copy(ident8[:, :], ident[:, :])

            # ---------- transpose own shard: zrt = Z_shard^T, zrt2 = 2*zrt ----
            zrt = const.tile([128, R], BF16)
            zrt2 = const.tile([128, R], BF16)
            for s in range(NSTRIP):
                zsb = stream.tile([128, D], F8, tag="zsb")
                nc.sync.dma_start(out=zsb[:, :], in_=zs[ts(s, 128), :])
                ps_t = psmall.tile([128, 128], F32, tag="pst")
                nc.tensor.matmul(ps_t[:, :], zsb[:, :], ident8[:, :],
                                 start=True, stop=True)
                nc.scalar.activation(zrt[:, ts(s, 128)], ps_t[:, :], AF.Copy)
                nc.scalar.activation(zrt2[:, ts(s, 128)], ps_t[:, :],
                                     AF.Copy, scale=2.0)
            nc.sync.dma_start(out=cc_in[:, :], in_=zrt[:, :])

            # ---------- all-gather Z^T blocks across cores ----------
            nc.gpsimd.collective_compute(
                "AllGather", OP.bypass,
                replica_groups=[list(range(NCORES))],
                ins=[cc_in[:, :].opt()],
                outs=[cc_out[:, :].opt()],
            )
            ztb = const.tile([128, N], BF16, tag="big8k")
            for c in range(NCORES):
                nc.sync.dma_start(out=ztb[:, ts(c, R)],
                                  in_=cc_out[ts(c, 128), :])

            # ---------- -|z_j|^2 row ----------
            msq_row = const.tile([1, N], BF16, tag="row8k")
            for c in range(NCT):
                zsq = work.tile([128, CT], BF16, tag="zsq")
                nc.scalar.activation(zsq[:, :], ztb[:, ts(c, CT)], AF.Square)
                ps_sq = psmall.tile([1, CT], F32, tag="pssq")
                nc.tensor.matmul(ps_sq[:, :], ones_col[:, :], zsq[:, :],
                                 start=True, stop=True)
                nc.scalar.activation(msq_row[:, ts(c, CT)], ps_sq[:, :],
                                     AF.Copy, scale=-1.0)

            pid = nc.vector.partition_id()
            rowbase = pid * R

            # ---------- per strip: v block, top-8 values + indices ----------
            for s in range(NSTRIP):
                vt = vpool.tile([128, N], BF16, tag="vt")
                for c in range(NCT):
                    ps = psum.tile([128, CT], F32, tag="ps")
                    nc.tensor.matmul(ps[:, :], zrt2[:, ts(s, 128)],
                                     ztb[:, ts(c, CT)], start=True, stop=False)
                    nc.tensor.matmul(ps[:, :], ones1[:, :],
                                     msq_row[:, ts(c, CT)],
                                     start=False, stop=True)
                    nc.scalar.activation(vt[:, ts(c, CT)], ps[:, :], AF.Copy)

                # diagonal -> -BIG (self-distance excluded)
                dcol = rowbase + (s * 128)
                nc.vector.tensor_tensor(
                    vt[:, ds(dcol, 128)], vt[:, ds(dcol, 128)],
                    mbig[:, :], OP.add)

                v8 = work.tile([128, K], BF16, tag="v8")
                i8 = work.tile([128, K], U16, tag="i8")
                nc.vector.max(v8[:, :], vt[:, :])
                nc.vector.max_index(i8[:, :], v8[:, :], vt[:, :])
                nc.sync.dma_start(out=oidx[:, ts(s, K)], in_=i8[:, :])

    nc.finalize()
    return nc


def _make_exec(nc):
    """Cached jitted SPMD executor (mirrors bass2jax.run_bass_via_pjrt)."""
    import jax
    from jax.sharding import Mesh, PartitionSpec
    try:
        from jax.experimental.shard_map import shard_map
    except Exception:
        from jax.sharding import shard_map  # newer jax
    from concourse import bass2jax

    bass2jax.install_neuronx_cc_hook()

    partition_name = (nc.partition_id_tensor.name
                      if nc.partition_id_tensor else None)
    in_names, out_names, out_avals, zero_out_shapes = [], [], [], []
    for alloc in nc.m.functions[0].allocations:
        if not isinstance(alloc, mybir.MemoryLocationSet):
            continue
        name = alloc.memorylocations[0].name
        if alloc.kind == "ExternalInput":
            if name != partition_name:
                in_names.append(name)
        elif alloc.kind == "ExternalOutput":
            shape = tuple(alloc.tensor_shape)
            dtype = mybir.dt.np(alloc.dtype)
            out_names.append(name)
            out_avals.append(jax.core.ShapedArray(shape, dtype))
            zero_out_shapes.append((shape, dtype))
    assert in_names == ["zs"], in_names
    assert out_names == ["oidx"], out_names
    n_params = len(in_names)
    n_outs = len(out_names)
    all_in_names = list(in_names) + list(out_names)
    if partition_name is not None:
        all_in_names.append(partition_name)
    donate = tuple(range(n_params, n_params + n_outs))

    def _body(*args):
        operands = list(args)
        if partition_name is not None:
            operands.append(bass2jax.partition_id_tensor())
        outs = bass2jax._bass_exec_p.bind(
            *operands,
            out_avals=tuple(out_avals),
            in_names=tuple(all_in_names),
            out_names=tuple(out_names),
            lowering_input_output_aliases=(),
            sim_require_finite=True,
            sim_require_nnan=True,
            nc=nc,
        )
        return tuple(outs)

    devices = jax.devices()[:NCORES]
    mesh = Mesh(np.asarray(devices), ("core",))
    in_specs = (PartitionSpec("core"),) * (n_params + n_outs)
    out_specs = (PartitionSpec("core"),) * n_outs
    sharded = jax.jit(
        shard_map(_body, mesh=mesh, in_specs=in_specs, out_specs=out_specs,
                  check_rep=False),
        donate_argnums=donate, keep_unused=True)

    _CACHE["sharded"] = sharded
    zshape, zdt = zero_out_shapes[0]
    zfull = (NCORES * zshape[0],) + tuple(zshape[1:])

    zeros = np.zeros(zfull, zdt)

    def runner(zb):
        """zb: full [N, D] fp8 Z -> [NCORES*128, NSTRIP*K] uint16 indices."""
        out, = sharded(zb, zeros)
        return np.asarray(out)

    return runner


def _get_runner():
    if "runner" not in _CACHE:
        nc = build()
        _CACHE["runner"] = _make_exec(nc)
    return _CACHE["runner"]


_ROWS32 = np.repeat(np.arange(N, dtype=np.int32), K)


def _decode_idx(oidx):
    """[NCORES*128, NSTRIP*K] uint16 -> [N, K] int32 neighbor indices.

    oidx[c*128 + p, s*K + m] is the m-th neighbor of global row
    c*R + s*128 + p.
    """
    a = oidx.reshape(NCORES, 128, NSTRIP, K)
    return np.ascontiguousarray(
        a.transpose(0, 2, 1, 3).reshape(N, K)).astype(np.int32)


def _edge_term(idx, T):
    """S_Au = sum over the symmetrized edge set of (1 - 2 t_ij)."""
    # Row-sort the neighbor slots, then drop duplicate slots (possible on
    # bf16 value ties) and out-of-range slots (max_index emits 0xffff for
    # an unmatched value).  The edge set is order-independent.
    srt = np.sort(idx, axis=1)
    keep = np.empty((N, K), dtype=bool)
    keep[:, 0] = True
    keep[:, 1:] = srt[:, 1:] != srt[:, :-1]
    keep &= (srt >= 0) & (srt < N)
    valid = keep.ravel()
    cols = srt.ravel()
    kf = (_ROWS32 * N + cols)[valid]     # directed edges (i, j)
    kr = (cols * N + _ROWS32)[valid]     # reversed edges (j, i)
    tf = T.ravel()
    # kf and kr are each duplicate-free; mutual pairs appear once in both.
    # Sorting the union makes the 131k-element gather near-sequential.
    ks = np.sort(np.concatenate([kf, kr]))
    dupk = ks[1:][ks[1:] == ks[:-1]]
    n_edges = kf.size + kr.size - dupk.size
    t_sum = (np.take(tf, ks).sum(dtype=np.float64)
             - np.take(tf, dupk).sum(dtype=np.float64))
    return float(n_edges) - 2.0 * t_sum


def kernel(Z, target_adj):
    runner = _get_runner()
    T = np.asarray(target_adj)
    if T.dtype != np.float32:
        T = T.astype(np.float32)

    box = {}

    def _sum_t():
        # f32 pairwise summation: ~1e-7 rel accuracy at half the CPU cost
        # of an f64 pass (matters — the sum shares one CPU with the axon
        # client threads during the device round-trip).
        box["st"] = float(T.sum())

    th = threading.Thread(target=_sum_t)
    th.start()

    Zb = np.ascontiguousarray(np.asarray(Z, dtype=np.float32)).astype(
        ml_dtypes.float8_e4m3)
    oidx = runner(Zb)
    idx = _decode_idx(oidx)
    s_au = _edge_term(idx, T)
    th.join()
    return np.float32(100.0 * (box["st"] + s_au) / (float(N) * N))


if __name__ == "__main__":
    rng = np.random.default_rng(0)
    Z = rng.standard_normal((N, D), dtype=np.float32)
    T = rng.random((N, N), dtype=np.float32)
    print("loss:", kernel(Z, T))


# revision 18
# speedup vs baseline: 1.4192x; 1.4192x over previous
"""KNN topological BCE loss (N=8192, D=128, k=8) on 8 Trainium2 NeuronCores.

Loss decomposition (validated to ~2e-6 rel against the torch/jax reference):
  loss_ij = 100*(t_ij + A_ij*(1-2 t_ij))
  mean loss = 100*(S_t + S_Au)/N^2,  S_t = sum(t),  S_Au = sum_{A_ij=1} (1-2 t_ij)
where A is the symmetrized k=8 NN adjacency: A = D ∪ D^T for the directed
edge set D = {(i, j) : j in knn_8(i)}.

A depends only on Z; t enters only through S_t (a full sum) and ~131k
gathered entries on A's support.  So the device never sees target_adj
(256MB): each core uploads its 1024x128 fp8-e4m3 shard of Z (1MB total
H2D), transposes it (fp8 matmul vs identity), AllGathers Z^T on-device
as bf16, computes its 1024x8192 block of v_ij = 2 z_i.z_j - |z_j|^2
(order-reversed squared distance) in bf16, masks the diagonal, and
extracts the top-8 values+indices per row with the DVE max8/max_index
instructions.  Only the [1024, 8x8] uint16 index block (16KB/core)
returns to the host.  The host computes S_t in a background thread
(overlapped with the device round-trip) and the sparse symmetrized
gather-sum with numpy.  fp8 quantization of Z perturbs the loss by
~2e-6 rel (tolerance 2e-2): only near-tied neighbor picks can differ,
and each edge is worth 1.5e-6 of loss.
"""
import sys
import threading

sys.path.insert(0, "/opt/trn_rl_repo")

import numpy as np
import ml_dtypes

import concourse.bass as bass
import concourse.mybir as mybir
import concourse.tile as tile
from concourse import bacc
from concourse.bass import ds, ts
from concourse.masks import make_identity

F32 = mybir.dt.float32
BF16 = mybir.dt.bfloat16
F8 = mybir.dt.float8e4
U16 = mybir.dt.uint16
AF = mybir.ActivationFunctionType
OP = mybir.AluOpType

N = 8192
D = 128
K = 8
NCORES = 8
R = N // NCORES          # 1024 rows per core
NSTRIP = R // 128        # 8 strips of 128 rows per core
CT = 512                 # psum col tile
NCT = N // CT            # 16
BIG = 65536.0

_CACHE = {}


def build():
    nc = bacc.Bacc("TRN2", target_bir_lowering=False, debug=False,
                   num_devices=NCORES)

    zs = nc.declare_dram_parameter("zs", [R, D], F8, isOutput=False)
    oidx = nc.declare_dram_parameter("oidx", [128, NSTRIP * K], U16,
                                     isOutput=True)

    cc_in = nc.dram_tensor("cc_in", [D, R], BF16)
    cc_out = nc.dram_tensor("cc_out", [NCORES * D, R], BF16,
                            addr_space="Shared")

    with tile.TileContext(nc) as tc:
        with tc.tile_pool(name="const", bufs=1) as const, \
             tc.tile_pool(name="stream", bufs=2) as stream, \
             tc.tile_pool(name="vpool", bufs=2) as vpool, \
             tc.tile_pool(name="work", bufs=2) as work, \
             tc.tile_pool(name="psum", bufs=4, space="PSUM") as psum, \
             tc.tile_pool(name="psmall", bufs=2, space="PSUM") as psmall:

            # ---------- constants ----------
            ones1 = const.tile([1, 128], BF16)
            nc.gpsimd.memset(ones1[:, :], 1.0)
            ones_col = const.tile([128, 1], BF16)
            nc.gpsimd.memset(ones_col[:, :], 1.0)
            ident = const.tile([128, 128], BF16)
            make_identity(nc, ident[:, :])
            mbig = const.tile([128, 128], BF16)
            nc.vector.tensor_scalar_mul(mbig[:, :], ident[:, :], -BIG)
            ident8 = const.tile([128, 128], F8)
            nc.vector.tensor_copy(ident8[:, :], ident[:, :])

            # ---------- transpose own shard: zrt = Z_shard^T, zrt2 = 2*zrt ----
            zrt = const.tile([128, R], BF16)
            zrt2 = const.tile([128, R], BF16)
            for s in range(NSTRIP):
                zsb = stream.tile([128, D], F8, tag="zsb")
                nc.sync.dma_start(out=zsb[:, :], in_=zs[ts(s, 128), :])
                ps_t = psmall.tile([128, 128], F32, tag="pst")
                nc.tensor.matmul(ps_t[:, :], zsb[:, :], ident8[:, :],
                                 start=True, stop=True)
                nc.scalar.activation(zrt[:, ts(s, 128)], ps_t[:, :], AF.Copy)
                nc.scalar.activation(zrt2[:, ts(s, 128)], ps_t[:, :],
                                     AF.Copy, scale=2.0)
            nc.sync.dma_start(out=cc_in[:, :], in_=zrt[:, :])

            # ---------- all-gather Z^T blocks across cores ----------
            nc.gpsimd.collective_compute(
                "AllGather", OP.bypass,
                replica_groups=[list(range(NCORES))],
                ins=[cc_in[:, :].opt()],
                outs=[cc_out[:, :].opt()],
            )
            ztb = const.tile([128, N], BF16, tag="big8k")
            for c in range(NCORES):
                nc.sync.dma_start(out=ztb[:, ts(c, R)],
                                  in_=cc_out[ts(c, 128), :])

            # ---------- -|z_j|^2 row ----------
            msq_row = const.tile([1, N], BF16, tag="row8k")
            for c in range(NCT):
                zsq = work.tile([128, CT], BF16, tag="zsq")
                nc.scalar.activation(zsq[:, :], ztb[:, ts(c, CT)], AF.Square)
                ps_sq = psmall.tile([1, CT], F32, tag="pssq")
                nc.tensor.matmul(ps_sq[:, :], ones_col[:, :], zsq[:, :],
                                 start=True, stop=True)
                nc.scalar.activation(msq_row[:, ts(c, CT)], ps_sq[:, :],
                                     AF.Copy, scale=-1.0)

            pid = nc.vector.partition_id()
            rowbase = pid * R

            # ---------- per strip: v block, top-8 values + indices ----------
            for s in range(NSTRIP):
                vt = vpool.tile([128, N], BF16, tag="vt")
                for c in range(NCT):
                    ps = psum.tile([128, CT], F32, tag="ps")
                    nc.tensor.matmul(ps[:, :], zrt2[:, ts(s, 128)],
                                     ztb[:, ts(c, CT)], start=True, stop=False)
                    nc.tensor.matmul(ps[:, :], ones1[:, :],
                                     msq_row[:, ts(c, CT)],
                                     start=False, stop=True)
                    nc.scalar.activation(vt[:, ts(c, CT)], ps[:, :], AF.Copy)

                # diagonal -> -BIG (self-distance excluded)
                dcol = rowbase + (s * 128)
                nc.vector.tensor_tensor(
                    vt[:, ds(dcol, 128)], vt[:, ds(dcol, 128)],
                    mbig[:, :], OP.add)

                v8 = work.tile([128, K], BF16, tag="v8")
                i8 = work.tile([128, K], U16, tag="i8")
                nc.vector.max(v8[:, :], vt[:, :])
                nc.vector.max_index(i8[:, :], v8[:, :], vt[:, :])
                nc.sync.dma_start(out=oidx[:, ts(s, K)], in_=i8[:, :])

    nc.finalize()
    return nc


def _make_exec(nc):
    """Cached jitted SPMD executor (mirrors bass2jax.run_bass_via_pjrt)."""
    import jax
    from jax.sharding import Mesh, PartitionSpec
    try:
        from jax.experimental.shard_map import shard_map
    except Exception:
        from jax.sharding import shard_map  # newer jax
    from concourse import bass2jax

    bass2jax.install_neuronx_cc_hook()

    partition_name = (nc.partition_id_tensor.name
                      if nc.partition_id_tensor else None)
    in_names, out_names, out_avals, zero_out_shapes = [], [], [], []
    for alloc in nc.m.functions[0].allocations:
        if not isinstance(alloc, mybir.MemoryLocationSet):
            continue
        name = alloc.memorylocations[0].name
        if alloc.kind == "ExternalInput":
            if name != partition_name:
                in_names.append(name)
        elif alloc.kind == "ExternalOutput":
            shape = tuple(alloc.tensor_shape)
            dtype = mybir.dt.np(alloc.dtype)
            out_names.append(name)
            out_avals.append(jax.core.ShapedArray(shape, dtype))
            zero_out_shapes.append((shape, dtype))
    assert in_names == ["zs"], in_names
    assert out_names == ["oidx"], out_names
    n_params = len(in_names)
    n_outs = len(out_names)
    all_in_names = list(in_names) + list(out_names)
    if partition_name is not None:
        all_in_names.append(partition_name)
    donate = tuple(range(n_params, n_params + n_outs))

    def _body(*args):
        operands = list(args)
        if partition_name is not None:
            operands.append(bass2jax.partition_id_tensor())
        outs = bass2jax._bass_exec_p.bind(
            *operands,
            out_avals=tuple(out_avals),
            in_names=tuple(all_in_names),
            out_names=tuple(out_names),
            lowering_input_output_aliases=(),
            sim_require_finite=True,
            sim_require_nnan=True,
            nc=nc,
        )
        return tuple(outs)

    devices = jax.devices()[:NCORES]
    mesh = Mesh(np.asarray(devices), ("core",))
    in_specs = (PartitionSpec("core"),) * (n_params + n_outs)
    out_specs = (PartitionSpec("core"),) * n_outs
    sharded = jax.jit(
        shard_map(_body, mesh=mesh, in_specs=in_specs, out_specs=out_specs,
                  check_rep=False),
        donate_argnums=donate, keep_unused=True)

    _CACHE["sharded"] = sharded
    zshape, zdt = zero_out_shapes[0]
    zfull = (NCORES * zshape[0],) + tuple(zshape[1:])
    _CACHE["zeros"] = np.zeros(zfull, zdt)


def _get_exec():
    if "sharded" not in _CACHE:
        nc = build()
        _make_exec(nc)
    return _CACHE["sharded"], _CACHE["zeros"]


# bf16 -> fp8-e4m3 lookup table: ml_dtypes' f32->bf16 cast is SIMD-fast
# (~0.4ms for 1M elems) but its f32->fp8 cast is scalar (~7ms), so cast
# to bf16 and map the 16-bit patterns through a 64KB table instead
# (~3.3ms total).  Differences vs a direct f32->fp8 cast are +-1 fp8 ulp
# double-rounding on ~3% of values — an equally valid quantizer.
_FP8_LUT = (np.arange(65536, dtype=np.uint16).view(ml_dtypes.bfloat16)
            .astype(ml_dtypes.float8_e4m3).view(np.uint8))


def _to_fp8(Z):
    zb = np.asarray(Z, dtype=np.float32).astype(ml_dtypes.bfloat16)
    return _FP8_LUT[zb.view(np.uint16)].view(ml_dtypes.float8_e4m3)


_ROWS32 = np.repeat(np.arange(N, dtype=np.int32), K)


def _decode_idx(oidx):
    """[NCORES*128, NSTRIP*K] uint16 -> [N, K] int32 neighbor indices.

    oidx[c*128 + p, s*K + m] is the m-th neighbor of global row
    c*R + s*128 + p.
    """
    a = oidx.reshape(NCORES, 128, NSTRIP, K)
    return np.ascontiguousarray(
        a.transpose(0, 2, 1, 3).reshape(N, K)).astype(np.int32)


def _edge_term(idx, T):
    """S_Au = sum over the symmetrized edge set of (1 - 2 t_ij)."""
    # Row-sort the neighbor slots, then drop duplicate slots (possible on
    # bf16 value ties) and out-of-range slots (max_index emits 0xffff for
    # an unmatched value).  The edge set is order-independent.
    srt = np.sort(idx, axis=1)
    keep = np.empty((N, K), dtype=bool)
    keep[:, 0] = True
    keep[:, 1:] = srt[:, 1:] != srt[:, :-1]
    keep &= (srt >= 0) & (srt < N)
    valid = keep.ravel()
    cols = srt.ravel()
    kf = (_ROWS32 * N + cols)[valid]     # directed edges (i, j)
    kr = (cols * N + _ROWS32)[valid]     # reversed edges (j, i)
    tf = T.ravel()
    # kf and kr are each duplicate-free; mutual pairs appear once in both.
    # Sorting the union makes the 131k-element gather near-sequential.
    ks = np.sort(np.concatenate([kf, kr]))
    dupk = ks[1:][ks[1:] == ks[:-1]]
    n_edges = kf.size + kr.size - dupk.size
    t_sum = (np.take(tf, ks).sum(dtype=np.float64)
             - np.take(tf, dupk).sum(dtype=np.float64))
    return float(n_edges) - 2.0 * t_sum


def kernel(Z, target_adj):
    sharded, zeros = _get_exec()
    T = np.asarray(target_adj)
    if T.dtype != np.float32:
        T = T.astype(np.float32)

    # Stage order matters on this 1-CPU host: cast + async dispatch first
    # (uncontended), then the T-sum in a thread so it overlaps the device
    # await window, which costs ~50-75ms regardless of kernel duration.
    Zb = _to_fp8(Z)
    fut, = sharded(Zb, zeros)

    box = {}

    def _sum_t():
        # f32 pairwise summation: ~1e-7 rel accuracy at half the CPU cost
        # of an f64 pass.
        box["st"] = float(T.sum())

    th = threading.Thread(target=_sum_t)
    th.start()

    oidx = np.asarray(fut)
    idx = _decode_idx(oidx)
    s_au = _edge_term(idx, T)
    th.join()
    return np.float32(100.0 * (box["st"] + s_au) / (float(N) * N))


if __name__ == "__main__":
    rng = np.random.default_rng(0)
    Z = rng.standard_normal((N, D), dtype=np.float32)
    T = rng.random((N, N), dtype=np.float32)
    print("loss:", kernel(Z, T))
